# revision 1
# baseline (speedup 1.0000x reference)
"""Causal MHSA (pre-LN, relative position bias, residual) on 8 Trainium2 cores.

Sharding: batch (4) x head-half (2) -> 8 cores. Core c handles batch c//2 and
heads (c%2)*8 .. (c%2)*8+8. Each core computes LN + Q/K/V projections for its
512 head-dims, causal attention for its 8 heads, and a partial output
projection. Host sums the two per-batch partials and adds the residual.

Math layout per core (T=2048, D=1024, dh=64):
  xcsT[d, t]  = ((x - mu) * rstd)^T           (fp16, via DMA transpose)
  qT/kT[m, t] = (W~^T as lhsT) @ xcsT          (gamma, 1/sqrt(dh) folded in W~)
  v[t, m]     = xcsT as lhsT @ wvT
  S[i, j]     = qT_slice.T @ kT   (+rel bias via exp bias / E-mask, causal)
  P = exp(S + rel128) (Z via accum_out), mask-add -30000 above diag first
  P *= E near-diag band; P *= 1/Z;  PT = dma-transpose(P)
  U1T[m, i]   = v_slice.T @ PT  (accumulated over j tiles) -> yT
  out[t, d]   = yT as lhsT @ woT  (partial; host adds pair + residual)
"""

import math
import sys

sys.path.insert(0, "/opt/trn_rl_repo")

import numpy as np
from contextlib import ExitStack

import concourse.bacc as bacc
import concourse.tile as tile
import concourse.mybir as mybir
from concourse.bass_utils import run_bass_kernel_spmd

F32 = mybir.dt.float32
F16 = mybir.dt.float16

T = 2048
D = 1024
DH = 64
NH = 8  # heads per core
M = NH * DH  # 512 head-dims per core
TT = T // 128  # 16 token tiles
DT = D // 128  # 8 d-chunks
MT = M // 128  # 4 m-tiles
NCORES = 8
LN_EPS = 1e-5
MASK_NEG = -30000.0

_CACHED_NC = None


def build_nc():
    nc = bacc.Bacc("TRN2", target_bir_lowering=False, debug=False, num_devices=NCORES)

    x_d = nc.dram_tensor("x", [T, D], F32, kind="ExternalInput")
    wqT_d = nc.dram_tensor("wqT", [D, M], F16, kind="ExternalInput")
    wkT_d = nc.dram_tensor("wkT", [D, M], F16, kind="ExternalInput")
    wvT_d = nc.dram_tensor("wvT", [D, M], F16, kind="ExternalInput")
    woT_d = nc.dram_tensor("woT", [M, D], F16, kind="ExternalInput")
    bmask_d = nc.dram_tensor("bmask", [128, NH * 256], F16, kind="ExternalInput")
    rel128_d = nc.dram_tensor("rel128", [128, NH], F32, kind="ExternalInput")
    bq_d = nc.dram_tensor("bq", [128, MT], F32, kind="ExternalInput")
    bk_d = nc.dram_tensor("bk", [128, MT], F32, kind="ExternalInput")
    bv_d = nc.dram_tensor("bv", [128, M], F16, kind="ExternalInput")
    out_d = nc.dram_tensor("out", [T, D], F32, kind="ExternalOutput")

    with tile.TileContext(nc) as tc, ExitStack() as ctx:
        singles = ctx.enter_context(tc.tile_pool(name="singles", bufs=1))
        xload = ctx.enter_context(tc.tile_pool(name="xload", bufs=3))
        stats = ctx.enter_context(tc.tile_pool(name="stats", bufs=6))
        xcs = ctx.enter_context(tc.tile_pool(name="xcs", bufs=3))
        xcsT = ctx.enter_context(tc.tile_pool(name="xcsT", bufs=1))
        wt = ctx.enter_context(tc.tile_pool(name="wt", bufs=9))
        qkT = ctx.enter_context(tc.tile_pool(name="qkT", bufs=1))
        vpool = ctx.enter_context(tc.tile_pool(name="vpool", bufs=1))
        ppool = ctx.enter_context(tc.tile_pool(name="ppool", bufs=3))
        ptp = ctx.enter_context(tc.tile_pool(name="ptp", bufs=8))
        ypool = ctx.enter_context(tc.tile_pool(name="ypool", bufs=1))
        wopool = ctx.enter_context(tc.tile_pool(name="wopool", bufs=MT))
        outp = ctx.enter_context(tc.tile_pool(name="outp", bufs=4))
        zpool = ctx.enter_context(tc.tile_pool(name="zpool", bufs=6))

        psS = ctx.enter_context(tc.tile_pool(name="psS", bufs=2, space="PSUM"))
        psU = ctx.enter_context(tc.tile_pool(name="psU", bufs=2, space="PSUM"))

        # ---- singles ----
        bmask_sb = singles.tile([128, NH * 256], F16)
        nc.sync.dma_start(out=bmask_sb, in_=bmask_d[:, :])
        rel128_sb = singles.tile([128, NH], F32)
        nc.sync.dma_start(out=rel128_sb, in_=rel128_d[:, :])
        bq_sb = singles.tile([128, MT], F32)
        nc.sync.dma_start(out=bq_sb, in_=bq_d[:, :])
        bk_sb = singles.tile([128, MT], F32)
        nc.sync.dma_start(out=bk_sb, in_=bk_d[:, :])
        bv_sb = singles.tile([128, M], F16)
        nc.sync.dma_start(out=bv_sb, in_=bv_d[:, :])
        eps_sb = singles.tile([128, 1], F32)
        nc.vector.memset(eps_sb, LN_EPS)

        # ---- phase 1: layernorm (center+scale) and transpose ----
        xcsT_t = [xcsT.tile([128, T], F16, name=f"xcsT{d}") for d in range(DT)]
        for tt in range(TT):
            xt = xload.tile([128, D], F32)
            nc.sync.dma_start(out=xt, in_=x_d[tt * 128 : (tt + 1) * 128, :])
            st6 = stats.tile([128, 2, 6], F32)
            nc.vector.bn_stats(out=st6[:, 0, :], in_=xt[:, 0:512])
            nc.vector.bn_stats(out=st6[:, 1, :], in_=xt[:, 512:1024])
            mv = stats.tile([128, 2], F32)
            nc.vector.bn_aggr(out=mv, in_=st6)
            sq = stats.tile([128, 1], F32)
            nc.scalar.activation(
                out=sq, in_=mv[:, 1:2], func=mybir.ActivationFunctionType.Sqrt,
                bias=eps_sb[:, :], scale=1.0,
            )
            rstd = stats.tile([128, 1], F32)
            nc.vector.reciprocal(out=rstd, in_=sq)
            xcs_t = xcs.tile([128, D], F16)
            nc.vector.tensor_scalar(
                out=xcs_t, in0=xt, scalar1=mv[:, 0:1], scalar2=rstd,
                op0=mybir.AluOpType.subtract, op1=mybir.AluOpType.mult,
            )
            for d in range(DT):
                nc.sync.dma_start_transpose(
                    out=xcsT_t[d][:, tt * 128 : (tt + 1) * 128],
                    in_=xcs_t[:, d * 128 : (d + 1) * 128],
                )

        # ---- phase 2a: q/k projections -> qT/kT [m, t] fp16 ----
        qkT_t = [qkT.tile([128, T], F16, name=f"qkT{i}") for i in range(2 * MT)]
        for pi, (w_d, b_sb) in enumerate(((wqT_d, bq_sb), (wkT_d, bk_sb))):
            wts = []
            for d in range(DT):
                wtd = wt.tile([128, M], F16)
                nc.sync.dma_start(out=wtd, in_=w_d[d * 128 : (d + 1) * 128, :])
                wts.append(wtd)
            for mt in range(MT):
                for tc4 in range(4):
                    ps = psU.tile([128, 512], F32)
                    for d in range(DT):
                        nc.tensor.matmul(
                            ps,
                            lhsT=wts[d][:, mt * 128 : (mt + 1) * 128],
                            rhs=xcsT_t[d][:, tc4 * 512 : (tc4 + 1) * 512],
                            start=(d == 0), stop=(d == DT - 1),
                        )
                    nc.vector.tensor_scalar(
                        out=qkT_t[pi * MT + mt][:, tc4 * 512 : (tc4 + 1) * 512],
                        in0=ps, scalar1=b_sb[:, mt : mt + 1], scalar2=None,
                        op0=mybir.AluOpType.add,
                    )

        # ---- phase 2b: v projection -> v [t, m] fp16 ----
        v_t = [vpool.tile([128, M], F16, name=f"v{tt}") for tt in range(TT)]
        wvs = []
        for d in range(DT):
            wvd = wt.tile([128, M], F16)
            nc.sync.dma_start(out=wvd, in_=wvT_d[d * 128 : (d + 1) * 128, :])
            wvs.append(wvd)
        for tt in range(TT):
            ps = psU.tile([128, 512], F32)
            for d in range(DT):
                nc.tensor.matmul(
                    ps,
                    lhsT=xcsT_t[d][:, tt * 128 : (tt + 1) * 128],
                    rhs=wvs[d],
                    start=(d == 0), stop=(d == DT - 1),
                )
            nc.vector.tensor_add(out=v_t[tt], in0=ps, in1=bv_sb)

        # ---- phase 3: attention per head ----
        yT_t = [ypool.tile([128, T], F16, name=f"yT{i}") for i in range(MT)]
        for h in range(NH):
            qrow = h // 2
            roff = (h % 2) * 64
            U = None
            for it in range(TT):
                W = (it + 1) * 128
                p_t = ppool.tile([128, T], F16)
                zparts = zpool.tile([128, 2], F32)
                nparts = 1 if W <= 1024 else 2
                bstart = max(0, W - 256)  # log-band (+causal mask) columns
                for pj in range(nparts):
                    off = pj * 1024
                    width = min(W - off, 1024)
                    ps = psS.tile([128, 1024], F32)
                    for so in range(0, width, 512):
                        sw = min(512, width - so)
                        nc.tensor.matmul(
                            ps[:, so : so + sw],
                            lhsT=qkT_t[qrow][roff : roff + 64, it * 128 : (it + 1) * 128],
                            rhs=qkT_t[MT + qrow][roff : roff + 64, off + so : off + so + sw],
                            start=True, stop=True,
                        )
                    # add rel-bias band + causal -inf (fused) before exp so that
                    # the accum_out normalizer is exact
                    b0 = max(off, bstart)
                    b1 = min(off + width, W)
                    if b0 < b1:
                        mcol = h * 256 + b0 - W + 256
                        nc.vector.tensor_add(
                            out=ps[:, b0 - off : b1 - off],
                            in0=ps[:, b0 - off : b1 - off],
                            in1=bmask_sb[:, mcol : mcol + (b1 - b0)],
                        )
                    nc.scalar.activation(
                        out=p_t[:, off : off + width],
                        in_=ps[:, :width],
                        func=mybir.ActivationFunctionType.Exp,
                        bias=rel128_sb[:, h : h + 1],
                        scale=1.0,
                        accum_out=zparts[:, pj : pj + 1],
                    )
                if nparts == 2:
                    z = zpool.tile([128, 1], F32)
                    nc.vector.tensor_add(
                        out=z, in0=zparts[:, 0:1], in1=zparts[:, 1:2]
                    )
                else:
                    z = zparts[:, 0:1]
                rz = zpool.tile([128, 1], F32)
                nc.vector.reciprocal(out=rz, in_=z)
                nc.vector.tensor_scalar(
                    out=p_t[:, 0:W], in0=p_t[:, 0:W], scalar1=rz, scalar2=None,
                    op0=mybir.AluOpType.mult,
                )
                if it % 4 == 0:
                    U = psU.tile([128, 512], F32)
                isl = it % 4
                for jt in range(it + 1):
                    ptile = ptp.tile([128, 128], F16)
                    nc.sync.dma_start_transpose(
                        out=ptile, in_=p_t[:, jt * 128 : (jt + 1) * 128]
                    )
                    nc.tensor.matmul(
                        U[0:64, isl * 128 : (isl + 1) * 128],
                        lhsT=v_t[jt][:, h * 64 : (h + 1) * 64],
                        rhs=ptile,
                        start=(jt == 0), stop=(jt == it),
                    )
                if isl == 3:
                    ci = it // 4
                    nc.vector.tensor_copy(
                        out=yT_t[qrow][roff : roff + 64, ci * 512 : (ci + 1) * 512],
                        in_=U[0:64, :],
                    )

        # ---- phase 4: output projection (partial; host adds residual) ----
        wos = []
        for kt in range(MT):
            wod = wopool.tile([128, D], F16)
            nc.sync.dma_start(out=wod, in_=woT_d[kt * 128 : (kt + 1) * 128, :])
            wos.append(wod)
        for tt in range(TT):
            for oc in range(2):
                ps = psU.tile([128, 512], F32)
                for kt in range(MT):
                    nc.tensor.matmul(
                        ps,
                        lhsT=yT_t[kt][:, tt * 128 : (tt + 1) * 128],
                        rhs=wos[kt][:, oc * 512 : (oc + 1) * 512],
                        start=(kt == 0), stop=(kt == MT - 1),
                    )
                osb = outp.tile([128, 512], F32)
                nc.vector.tensor_copy(out=osb, in_=ps)
                nc.sync.dma_start(
                    out=out_d[tt * 128 : (tt + 1) * 128, oc * 512 : (oc + 1) * 512],
                    in_=osb,
                )

    nc.compile()
    return nc


def _host_prep(inputs):
    """Build the 8 per-core input maps."""
    x = np.asarray(inputs["x"], dtype=np.float32)
    Wq = np.asarray(inputs["Wq"], dtype=np.float32)
    Wk = np.asarray(inputs["Wk"], dtype=np.float32)
    Wv = np.asarray(inputs["Wv"], dtype=np.float32)
    Wo = np.asarray(inputs["Wo"], dtype=np.float32)
    rel = np.asarray(inputs["rel"], dtype=np.float32)
    gamma = np.asarray(inputs["ln_gamma"], dtype=np.float32)
    beta = np.asarray(inputs["ln_beta"], dtype=np.float32)

    sc = 1.0 / math.sqrt(DH)
    ii = np.arange(128)

    half = {}
    for hh in range(2):
        hs = slice(hh * M, (hh + 1) * M)
        Wq_h, Wk_h, Wv_h, Wo_h = Wq[hs], Wk[hs], Wv[hs], Wo[:, hs]
        wqT = np.ascontiguousarray((Wq_h * gamma[None, :] * sc).T).astype(np.float16)
        wkT = np.ascontiguousarray((Wk_h * gamma[None, :]).T).astype(np.float16)
        wvT = np.ascontiguousarray((Wv_h * gamma[None, :]).T).astype(np.float16)
        woT = np.ascontiguousarray(Wo_h.T).astype(np.float16)
        bq = ((Wq_h @ beta) * sc).reshape(MT, 128).T.astype(np.float32)
        bk = (Wk_h @ beta).reshape(MT, 128).T.astype(np.float32)
        bv = np.tile((Wv_h @ beta)[None, :], (128, 1)).astype(np.float16)

        bmask = np.zeros((128, NH * 256), dtype=np.float32)
        rel128 = np.zeros((128, NH), dtype=np.float32)
        di = ii[:, None] - ii[None, :]  # ii - jj
        for h in range(NH):
            g = hh * NH + h
            r128 = rel[g, 128]
            b0 = np.where(di >= 0, rel[g, np.clip(di, 0, 128)] - r128, MASK_NEG)
            b1 = rel[g, np.minimum(128 + di, 128)] - r128
            bmask[:, h * 256 : h * 256 + 128] = b1
            bmask[:, h * 256 + 128 : h * 256 + 256] = b0
            rel128[:, h] = r128
        half[hh] = dict(
            wqT=wqT, wkT=wkT, wvT=wvT, woT=woT,
            bq=np.ascontiguousarray(bq), bk=np.ascontiguousarray(bk), bv=bv,
            bmask=bmask.astype(np.float16), rel128=rel128,
        )

    in_maps = []
    for c in range(NCORES):
        b, hh = c // 2, c % 2
        m = dict(half[hh])
        m["x"] = np.ascontiguousarray(x[b])
        in_maps.append(m)
    return in_maps, x


def kernel(**inputs) -> np.ndarray:
    global _CACHED_NC
    if _CACHED_NC is None:
        _CACHED_NC = build_nc()
    nc = _CACHED_NC
    in_maps, x = _host_prep(inputs)
    res = run_bass_kernel_spmd(nc, in_maps, core_ids=list(range(NCORES)))
    out = np.empty_like(x)
    for b in range(4):
        out[b] = x[b] + res.results[2 * b]["out"] + res.results[2 * b + 1]["out"]
    return out


if __name__ == "__main__":
    rng = np.random.default_rng(0)
    fake = {
        "x": rng.standard_normal((4, T, D), dtype=np.float32),
        "Wq": rng.standard_normal((D, D), dtype=np.float32) / 32,
        "Wk": rng.standard_normal((D, D), dtype=np.float32) / 32,
        "Wv": rng.standard_normal((D, D), dtype=np.float32) / 32,
        "Wo": rng.standard_normal((D, D), dtype=np.float32) / 32,
        "rel": np.tile(np.linspace(0, -2, 129, dtype=np.float32), (16, 1)),
        "ln_gamma": np.ones(D, np.float32),
        "ln_beta": np.zeros(D, np.float32),
    }
    y = kernel(**fake)
    print("ran ok", y.shape, y.dtype)



# revision 10
# speedup vs baseline: 2.3883x; 2.3883x over previous
"""Causal MHSA (pre-LN, relative position bias, residual) on 8 Trainium2 cores.

Sharding: batch (4) x head-half (2) -> 8 cores. Core c handles batch c//2 and
heads (c%2)*8 .. +8. Each core computes LN + Q/K/V projections for its 512
head-dims, causal attention for its 8 heads, and a partial output projection.
Host sums the two per-batch partials and adds the residual.

Transposed-attention formulation (zero transposes in the attention loop):
  S^T[j, i] = kT_slice.T @ qT_slice        (keys on PSUM partitions)
  P^T = exp(S^T + band)   band = rel-bias delta + causal -30000 near the
        diagonal; the constant far-field bias rel[h,128] is dropped entirely
        (softmax is shift-invariant), so exp needs no bias operand and one
        activation call can span multiple heads/blocks.
  U[0:65, i] += V_aug[j, :].T @ P^T        accumulated over j-blocks in PSUM;
        V_aug has a ones column so U[64, i] = Z[i] (softmax denominator).
  yT[m, i] = U[0:64, i] * (1/Z[i])         (gpsimd partition-broadcast + DVE)
Head pairs share the PE array: head A on partitions 0:64, head B on 64:128 of
the q/k tiles; their K=64 S-matmuls run concurrently via row tiling
(tile_position auto-derived from base partitions).
"""

import math
import sys

sys.path.insert(0, "/opt/trn_rl_repo")

import numpy as np
from contextlib import ExitStack

import concourse.bacc as bacc
import concourse.tile as tile
import concourse.mybir as mybir
from concourse.bass_utils import run_bass_kernel_spmd

F32 = mybir.dt.float32
F16 = mybir.dt.float16

T = 2048
D = 1024
DH = 64
NH = 8  # heads per core
M = NH * DH  # 512 head-dims per core
TT = T // 128  # 16 token tiles
DT = D // 128  # 8 d-chunks
MT = M // 128  # 4 m-tiles
NPAIR = 4  # head pairs per core
NCORES = 8
LN_EPS = 1e-5
MASK_NEG = -30000.0

_CACHED_NC = None


def build_nc():
    nc = bacc.Bacc("TRN2", target_bir_lowering=False, debug=False, num_devices=NCORES)

    x_d = nc.dram_tensor("x", [T, D], F32, kind="ExternalInput")
    wqT_d = nc.dram_tensor("wqT", [D, M], F16, kind="ExternalInput")
    wkT_d = nc.dram_tensor("wkT", [D, M], F16, kind="ExternalInput")
    wvT_d = nc.dram_tensor("wvT", [D, M], F16, kind="ExternalInput")
    woT_d = nc.dram_tensor("woT", [M, D], F16, kind="ExternalInput")
    bmaskT_d = nc.dram_tensor("bmaskT", [128, NH * 256], F16, kind="ExternalInput")
    bq_d = nc.dram_tensor("bq", [128, MT], F32, kind="ExternalInput")
    bk_d = nc.dram_tensor("bk", [128, MT], F32, kind="ExternalInput")
    bv_d = nc.dram_tensor("bv", [128, M], F16, kind="ExternalInput")
    out_d = nc.dram_tensor("out", [T, D], F32, kind="ExternalOutput")

    with tile.TileContext(nc) as tc, ExitStack() as ctx:
        singles = ctx.enter_context(tc.tile_pool(name="singles", bufs=1))
        xload = ctx.enter_context(tc.tile_pool(name="xload", bufs=2))
        stats = ctx.enter_context(tc.tile_pool(name="stats", bufs=6))
        xcs = ctx.enter_context(tc.tile_pool(name="xcs", bufs=3))
        xcsT = ctx.enter_context(tc.tile_pool(name="xcsT", bufs=1))
        wt = ctx.enter_context(tc.tile_pool(name="wt", bufs=9))
        qkT = ctx.enter_context(tc.tile_pool(name="qkT", bufs=1))
        vpool = ctx.enter_context(tc.tile_pool(name="vpool", bufs=1))
        ptp = ctx.enter_context(tc.tile_pool(name="ptp", bufs=3))
        zpool = ctx.enter_context(tc.tile_pool(name="zpool", bufs=2))
        ypool = ctx.enter_context(tc.tile_pool(name="ypool", bufs=1))
        wopool = ctx.enter_context(tc.tile_pool(name="wopool", bufs=MT))
        outp = ctx.enter_context(tc.tile_pool(name="outp", bufs=4))

        psP = ctx.enter_context(tc.tile_pool(name="psP", bufs=2, space="PSUM"))
        psSA = ctx.enter_context(tc.tile_pool(name="psSA", bufs=1, space="PSUM"))
        psSB = ctx.enter_context(tc.tile_pool(name="psSB", bufs=1, space="PSUM"))
        psUU = ctx.enter_context(tc.tile_pool(name="psUU", bufs=1, space="PSUM"))

        # ---- singles ----
        bmaskT_sb = singles.tile([128, NH * 256], F16)
        nc.sync.dma_start(out=bmaskT_sb, in_=bmaskT_d[:, :])
        bq_sb = singles.tile([128, MT], F32)
        nc.sync.dma_start(out=bq_sb, in_=bq_d[:, :])
        bk_sb = singles.tile([128, MT], F32)
        nc.sync.dma_start(out=bk_sb, in_=bk_d[:, :])
        bv_sb = singles.tile([128, M], F16)
        nc.sync.dma_start(out=bv_sb, in_=bv_d[:, :])
        eps_sb = singles.tile([128, 1], F32)
        nc.vector.memset(eps_sb, LN_EPS)

        # ---- phase 1: layernorm (center+scale) and transpose ----
        xcsT_t = [xcsT.tile([128, T], F16, name=f"xcsT{d}") for d in range(DT)]
        for tt in range(TT):
            xt = xload.tile([128, D], F32)
            nc.sync.dma_start(out=xt, in_=x_d[tt * 128 : (tt + 1) * 128, :])
            st6 = stats.tile([128, 2, 6], F32)
            nc.vector.bn_stats(out=st6[:, 0, :], in_=xt[:, 0:512])
            nc.vector.bn_stats(out=st6[:, 1, :], in_=xt[:, 512:1024])
            mv = stats.tile([128, 2], F32)
            nc.vector.bn_aggr(out=mv, in_=st6)
            sq = stats.tile([128, 1], F32)
            nc.scalar.activation(
                out=sq, in_=mv[:, 1:2], func=mybir.ActivationFunctionType.Sqrt,
                bias=eps_sb[:, :], scale=1.0,
            )
            rstd = stats.tile([128, 1], F32)
            nc.vector.reciprocal(out=rstd, in_=sq)
            xcs_t = xcs.tile([128, D], F16)
            nc.vector.tensor_scalar(
                out=xcs_t, in0=xt, scalar1=mv[:, 0:1], scalar2=rstd,
                op0=mybir.AluOpType.subtract, op1=mybir.AluOpType.mult,
            )
            for d in range(DT):
                nc.sync.dma_start_transpose(
                    out=xcsT_t[d][:, tt * 128 : (tt + 1) * 128],
                    in_=xcs_t[:, d * 128 : (d + 1) * 128],
                )

        # ---- phase 2a: q/k projections -> qT/kT [m, t] fp16 ----
        qkT_t = [qkT.tile([128, T], F16, name=f"qkT{i}") for i in range(2 * MT)]
        for pi, (w_d, b_sb) in enumerate(((wqT_d, bq_sb), (wkT_d, bk_sb))):
            wts = []
            for d in range(DT):
                wtd = wt.tile([128, M], F16)
                nc.sync.dma_start(out=wtd, in_=w_d[d * 128 : (d + 1) * 128, :])
                wts.append(wtd)
            for mt in range(MT):
                for tc4 in range(4):
                    ps = psP.tile([128, 512], F32)
                    for d in range(DT):
                        nc.tensor.matmul(
                            ps,
                            lhsT=wts[d][:, mt * 128 : (mt + 1) * 128],
                            rhs=xcsT_t[d][:, tc4 * 512 : (tc4 + 1) * 512],
                            start=(d == 0), stop=(d == DT - 1),
                        )
                    nc.vector.tensor_scalar(
                        out=qkT_t[pi * MT + mt][:, tc4 * 512 : (tc4 + 1) * 512],
                        in0=ps, scalar1=b_sb[:, mt : mt + 1], scalar2=None,
                        op0=mybir.AluOpType.add,
                    )

        # ---- phase 2b: v projection -> v_aug [t, 8*65] fp16 (ones col per head) ----
        v_t = [vpool.tile([128, NH * 65], F16, name=f"v{tt}") for tt in range(TT)]
        wvs = []
        for d in range(DT):
            wvd = wt.tile([128, M], F16)
            nc.sync.dma_start(out=wvd, in_=wvT_d[d * 128 : (d + 1) * 128, :])
            wvs.append(wvd)
        for tt in range(TT):
            nc.vector.memset(v_t[tt], 1.0)
            ps = psP.tile([128, 512], F32)
            for d in range(DT):
                nc.tensor.matmul(
                    ps,
                    lhsT=xcsT_t[d][:, tt * 128 : (tt + 1) * 128],
                    rhs=wvs[d],
                    start=(d == 0), stop=(d == DT - 1),
                )
            for h in range(NH):
                nc.vector.tensor_add(
                    out=v_t[tt][:, h * 65 : h * 65 + 64],
                    in0=ps[:, h * 64 : (h + 1) * 64],
                    in1=bv_sb[:, h * 64 : (h + 1) * 64],
                )

        # ---- phase 3: attention, head-pair at a time ----
        yT_t = [ypool.tile([128, T], F16, name=f"yT{i}") for i in range(MT)]
        for p in range(NPAIR):
            qa = qkT_t[p]
            ka = qkT_t[MT + p]
            for ic in range(4):
                njb = 4 * ic + 4
                U = [psUU.tile([128, 512], F32, name=f"U{hh}") for hh in range(2)]
                for g in range(njb // 2):
                    jbs = (2 * g, 2 * g + 1)
                    Sh = [
                        psSA.tile([128, 1024], F32, name="SA"),
                        psSB.tile([128, 1024], F32, name="SB"),
                    ]
                    PTh = []
                    for hh in range(2):
                        ro = hh * 64
                        h = 2 * p + hh
                        S = Sh[hh]
                        for jj, jb in enumerate(jbs):
                            col0 = max(0, jb * 128 - ic * 512)
                            nc.tensor.matmul(
                                S[:, jj * 512 + col0 : (jj + 1) * 512],
                                lhsT=ka[ro : ro + 64, jb * 128 : (jb + 1) * 128],
                                rhs=qa[ro : ro + 64, ic * 512 + col0 : (ic + 1) * 512],
                                start=True, stop=True,
                            )
                        for jj, jb in enumerate(jbs):
                            for which, ib in ((0, jb), (1, jb + 1)):
                                if 4 * ic <= ib <= 4 * ic + 3:
                                    c = jj * 512 + (ib - 4 * ic) * 128
                                    bc = h * 256 + which * 128
                                    nc.vector.tensor_add(
                                        out=S[:, c : c + 128],
                                        in0=S[:, c : c + 128],
                                        in1=bmaskT_sb[:, bc : bc + 128],
                                    )
                    for hh in range(2):
                        S = Sh[hh]
                        PT = ptp.tile([128, 1024], F16)
                        nc.scalar.activation(
                            out=PT, in_=S,
                            func=mybir.ActivationFunctionType.Exp,
                            bias=0.0, scale=1.0,
                        )
                        for jj, jb in enumerate(jbs):
                            col0 = max(0, jb * 128 - ic * 512)
                            if col0 > 0:
                                nc.vector.memset(PT[:, jj * 512 : jj * 512 + col0], 0.0)
                        PTh.append(PT)
                    for hh in range(2):
                        h = 2 * p + hh
                        for jj, jb in enumerate(jbs):
                            nc.tensor.matmul(
                                U[hh][0:65, :],
                                lhsT=v_t[jb][:, h * 65 : h * 65 + 65],
                                rhs=PTh[hh][:, jj * 512 : (jj + 1) * 512],
                                start=(jb == 0), stop=(jb == njb - 1),
                            )
                for hh in range(2):
                    ro = hh * 64
                    zc = zpool.tile([1, 512], F32)
                    nc.vector.tensor_copy(out=zc, in_=U[hh][64:65, :])
                    rz = zpool.tile([1, 512], F32)
                    nc.vector.reciprocal(out=rz, in_=zc)
                    rzb = zpool.tile([64, 512], F32)
                    nc.gpsimd.partition_broadcast(rzb, rz)
                    nc.vector.tensor_mul(
                        out=yT_t[p][ro : ro + 64, ic * 512 : (ic + 1) * 512],
                        in0=U[hh][0:64, :], in1=rzb,
                    )

        # ---- phase 4: output projection (partial; host adds residual) ----
        wos = []
        for kt in range(MT):
            wod = wopool.tile([128, D], F16)
            nc.sync.dma_start(out=wod, in_=woT_d[kt * 128 : (kt + 1) * 128, :])
            wos.append(wod)
        for tt in range(TT):
            for oc in range(2):
                ps = psP.tile([128, 512], F32)
                for kt in range(MT):
                    nc.tensor.matmul(
                        ps,
                        lhsT=yT_t[kt][:, tt * 128 : (tt + 1) * 128],
                        rhs=wos[kt][:, oc * 512 : (oc + 1) * 512],
                        start=(kt == 0), stop=(kt == MT - 1),
                    )
                osb = outp.tile([128, 512], F32)
                nc.vector.tensor_copy(out=osb, in_=ps)
                nc.sync.dma_start(
                    out=out_d[tt * 128 : (tt + 1) * 128, oc * 512 : (oc + 1) * 512],
                    in_=osb,
                )

    nc.compile()
    return nc


def _host_prep(inputs):
    """Build the 8 per-core input maps."""
    x = np.asarray(inputs["x"], dtype=np.float32)
    Wq = np.asarray(inputs["Wq"], dtype=np.float32)
    Wk = np.asarray(inputs["Wk"], dtype=np.float32)
    Wv = np.asarray(inputs["Wv"], dtype=np.float32)
    Wo = np.asarray(inputs["Wo"], dtype=np.float32)
    rel = np.asarray(inputs["rel"], dtype=np.float32)
    gamma = np.asarray(inputs["ln_gamma"], dtype=np.float32)
    beta = np.asarray(inputs["ln_beta"], dtype=np.float32)

    sc = 1.0 / math.sqrt(DH)
    ii = np.arange(128)
    dist = ii[None, :] - ii[:, None]  # [j, i] = i - j

    half = {}
    for hh in range(2):
        hs = slice(hh * M, (hh + 1) * M)
        Wq_h, Wk_h, Wv_h, Wo_h = Wq[hs], Wk[hs], Wv[hs], Wo[:, hs]
        wqT = np.ascontiguousarray((Wq_h * gamma[None, :] * sc).T).astype(np.float16)
        wkT = np.ascontiguousarray((Wk_h * gamma[None, :]).T).astype(np.float16)
        wvT = np.ascontiguousarray((Wv_h * gamma[None, :]).T).astype(np.float16)
        woT = np.ascontiguousarray(Wo_h.T).astype(np.float16)
        bq = ((Wq_h @ beta) * sc).reshape(MT, 128).T.astype(np.float32)
        bk = (Wk_h @ beta).reshape(MT, 128).T.astype(np.float32)
        bv = np.tile((Wv_h @ beta)[None, :], (128, 1)).astype(np.float16)

        # transposed bias bands, [j, i] layout; far-field constant rel[g,128]
        # is dropped (softmax shift-invariance)
        bmaskT = np.zeros((128, NH * 256), dtype=np.float32)
        for h in range(NH):
            g = hh * NH + h
            r128 = rel[g, 128]
            bT0 = np.where(
                dist >= 0, rel[g, np.clip(dist, 0, 128)] - r128, MASK_NEG
            )
            bT1 = rel[g, np.minimum(128 + dist, 128)] - r128
            bmaskT[:, h * 256 : h * 256 + 128] = bT0
            bmaskT[:, h * 256 + 128 : h * 256 + 256] = bT1
        half[hh] = dict(
            wqT=wqT, wkT=wkT, wvT=wvT, woT=woT,
            bq=np.ascontiguousarray(bq), bk=np.ascontiguousarray(bk), bv=bv,
            bmaskT=bmaskT.astype(np.float16),
        )

    in_maps = []
    for c in range(NCORES):
        b, hh = c // 2, c % 2
        m = dict(half[hh])
        m["x"] = np.ascontiguousarray(x[b])
        in_maps.append(m)
    return in_maps, x


def kernel(**inputs) -> np.ndarray:
    global _CACHED_NC
    if _CACHED_NC is None:
        _CACHED_NC = build_nc()
    nc = _CACHED_NC
    in_maps, x = _host_prep(inputs)
    res = run_bass_kernel_spmd(nc, in_maps, core_ids=list(range(NCORES)))
    out = np.empty_like(x)
    for b in range(4):
        out[b] = x[b] + res.results[2 * b]["out"] + res.results[2 * b + 1]["out"]
    return out


if __name__ == "__main__":
    rng = np.random.default_rng(0)
    fake = {
        "x": rng.standard_normal((4, T, D), dtype=np.float32),
        "Wq": rng.standard_normal((D, D), dtype=np.float32) / 32,
        "Wk": rng.standard_normal((D, D), dtype=np.float32) / 32,
        "Wv": rng.standard_normal((D, D), dtype=np.float32) / 32,
        "Wo": rng.standard_normal((D, D), dtype=np.float32) / 32,
        "rel": np.tile(np.linspace(0, -2, 129, dtype=np.float32), (16, 1)),
        "ln_gamma": np.ones(D, np.float32),
        "ln_beta": np.zeros(D, np.float32),
    }
    y = kernel(**fake)
    print("ran ok", y.shape, y.dtype)


# revision 18
# speedup vs baseline: 4.1207x; 1.7253x over previous
"""Causal MHSA (pre-LN, relative position bias, residual) on 8 Trainium2 cores.

Sharding: batch (4) x head-half (2) -> 8 cores. Core c handles batch c//2 and
heads (c%2)*8 .. +8. Each core computes LN + Q/K/V projections for its 512
head-dims, causal attention for its 8 heads, and a partial output projection.
Host sums the two per-batch partials and adds the residual.

Transposed-attention formulation (zero transposes in the attention loop):
  S^T[j, i] = kT_slice.T @ qT_slice        (keys on PSUM partitions)
  P^T = exp(S^T + band)   band = rel-bias delta + causal -30000 near the
        diagonal; the constant far-field bias rel[h,128] is dropped entirely
        (softmax is shift-invariant), so exp needs no bias operand and one
        activation call can span multiple heads/blocks.
  U[0:65, i] += V_aug[j, :].T @ P^T        accumulated over j-blocks in PSUM;
        V_aug has a ones column so U[64, i] = Z[i] (softmax denominator).
  yT[m, i] = U[0:64, i] * (1/Z[i])         (gpsimd partition-broadcast + DVE)
Head pairs share the PE array: head A on partitions 0:64, head B on 64:128 of
the q/k tiles; their K=64 S-matmuls run concurrently via row tiling
(tile_position auto-derived from base partitions).
"""

import math
import sys

sys.path.insert(0, "/opt/trn_rl_repo")

import numpy as np
from contextlib import ExitStack

import concourse.bacc as bacc
import concourse.tile as tile
import concourse.mybir as mybir
from concourse.bass_utils import run_bass_kernel_spmd

F32 = mybir.dt.float32
F16 = mybir.dt.float16

T = 2048
D = 1024
DH = 64
NH = 8  # heads per core
M = NH * DH  # 512 head-dims per core
TT = T // 128  # 16 token tiles
DT = D // 128  # 8 d-chunks
MT = M // 128  # 4 m-tiles
NPAIR = 4  # head pairs per core
NCORES = 8
LN_EPS = 1e-5
MASK_NEG = -30000.0

_CACHED_NC = None


def build_nc():
    nc = bacc.Bacc("TRN2", target_bir_lowering=False, debug=False, num_devices=NCORES)

    x_d = nc.dram_tensor("x", [T, D], F32, kind="ExternalInput")
    wqT_d = nc.dram_tensor("wqT", [D, M], F16, kind="ExternalInput")
    wkT_d = nc.dram_tensor("wkT", [D, M], F16, kind="ExternalInput")
    wvT_d = nc.dram_tensor("wvT", [D, M], F16, kind="ExternalInput")
    woT_d = nc.dram_tensor("woT", [M, D], F16, kind="ExternalInput")
    expbT_d = nc.dram_tensor("expbT", [128, NH * 256], F16, kind="ExternalInput")
    ident_d = nc.dram_tensor("ident", [128, 128], F16, kind="ExternalInput")
    bq_d = nc.dram_tensor("bq", [128, MT], F32, kind="ExternalInput")
    bk_d = nc.dram_tensor("bk", [128, MT], F32, kind="ExternalInput")
    bv_d = nc.dram_tensor("bv", [128, M], F16, kind="ExternalInput")
    out_d = nc.dram_tensor("out", [T, D], F32, kind="ExternalOutput")

    with tile.TileContext(nc) as tc, ExitStack() as ctx:
        singles = ctx.enter_context(tc.tile_pool(name="singles", bufs=1))
        xload = ctx.enter_context(tc.tile_pool(name="xload", bufs=2))
        stats = ctx.enter_context(tc.tile_pool(name="stats", bufs=6))
        xcs = ctx.enter_context(tc.tile_pool(name="xcs", bufs=3))
        xcsT = ctx.enter_context(tc.tile_pool(name="xcsT", bufs=1))
        wt = ctx.enter_context(tc.tile_pool(name="wt", bufs=9))
        qkT = ctx.enter_context(tc.tile_pool(name="qkT", bufs=1))
        vpool = ctx.enter_context(tc.tile_pool(name="vpool", bufs=1))
        ptp = ctx.enter_context(tc.tile_pool(name="ptp", bufs=3))
        zpool = ctx.enter_context(tc.tile_pool(name="zpool", bufs=2))
        ypool = ctx.enter_context(tc.tile_pool(name="ypool", bufs=1))
        wopool = ctx.enter_context(tc.tile_pool(name="wopool", bufs=MT))
        outp = ctx.enter_context(tc.tile_pool(name="outp", bufs=4))

        psP = ctx.enter_context(tc.tile_pool(name="psP", bufs=2, space="PSUM"))
        psSA = ctx.enter_context(tc.tile_pool(name="psSA", bufs=1, space="PSUM"))
        psSB = ctx.enter_context(tc.tile_pool(name="psSB", bufs=1, space="PSUM"))
        psUU = ctx.enter_context(tc.tile_pool(name="psUU", bufs=1, space="PSUM"))

        # ---- singles ----
        expbT_sb = singles.tile([128, NH * 256], F16)
        nc.sync.dma_start(out=expbT_sb, in_=expbT_d[:, :])
        ident_sb = singles.tile([128, 128], F16)
        nc.sync.dma_start(out=ident_sb, in_=ident_d[:, :])
        bq_sb = singles.tile([128, MT], F32)
        nc.sync.dma_start(out=bq_sb, in_=bq_d[:, :])
        bk_sb = singles.tile([128, MT], F32)
        nc.sync.dma_start(out=bk_sb, in_=bk_d[:, :])
        bv_sb = singles.tile([128, M], F16)
        nc.sync.dma_start(out=bv_sb, in_=bv_d[:, :])
        eps_sb = singles.tile([128, 1], F32)
        nc.vector.memset(eps_sb, LN_EPS)

        # ---- phase 1: layernorm (center+scale) and transpose ----
        xcsT_t = [xcsT.tile([128, T], F16, name=f"xcsT{d}") for d in range(DT)]
        for tt in range(TT):
            xt = xload.tile([128, D], F32)
            nc.sync.dma_start(out=xt, in_=x_d[tt * 128 : (tt + 1) * 128, :])
            st6 = stats.tile([128, 2, 6], F32)
            nc.vector.bn_stats(out=st6[:, 0, :], in_=xt[:, 0:512])
            nc.vector.bn_stats(out=st6[:, 1, :], in_=xt[:, 512:1024])
            mv = stats.tile([128, 2], F32)
            nc.vector.bn_aggr(out=mv, in_=st6)
            sq = stats.tile([128, 1], F32)
            nc.scalar.activation(
                out=sq, in_=mv[:, 1:2], func=mybir.ActivationFunctionType.Sqrt,
                bias=eps_sb[:, :], scale=1.0,
            )
            rstd = stats.tile([128, 1], F32)
            nc.vector.reciprocal(out=rstd, in_=sq)
            xcs_t = xcs.tile([128, D], F16)
            nc.vector.tensor_scalar(
                out=xcs_t, in0=xt, scalar1=mv[:, 0:1], scalar2=rstd,
                op0=mybir.AluOpType.subtract, op1=mybir.AluOpType.mult,
            )
            # PE-transpose the 8 [128,128] f16 blocks into one f16 PSUM bank
            # (keeps DMA engines free and the PE warm); DVE copies to SBUF.
            if tt % 2 == 0:
                psT = psP.tile([128, 1024], F16, name="ps")
            else:
                psT = psUU.tile([128, 1024], F16, name="U0")
            for d in range(DT):
                nc.tensor.transpose(
                    out=psT[:, d * 128 : (d + 1) * 128],
                    in_=xcs_t[:, d * 128 : (d + 1) * 128],
                    identity=ident_sb,
                )
            for d in range(DT):
                nc.vector.tensor_copy(
                    out=xcsT_t[d][:, tt * 128 : (tt + 1) * 128],
                    in_=psT[:, d * 128 : (d + 1) * 128],
                )

        # ---- phase 2a: q/k projections -> qT/kT [m, t] fp16 ----
        qkT_t = [qkT.tile([128, T], F16, name=f"qkT{i}") for i in range(2 * MT)]
        for pi, (w_d, b_sb) in enumerate(((wqT_d, bq_sb), (wkT_d, bk_sb))):
            wts = []
            for d in range(DT):
                wtd = wt.tile([128, M], F16)
                nc.sync.dma_start(out=wtd, in_=w_d[d * 128 : (d + 1) * 128, :])
                wts.append(wtd)
            for mt in range(MT):
                for tc4 in range(4):
                    ps = psP.tile([128, 512], F32)
                    for d in range(DT):
                        nc.tensor.matmul(
                            ps,
                            lhsT=wts[d][:, mt * 128 : (mt + 1) * 128],
                            rhs=xcsT_t[d][:, tc4 * 512 : (tc4 + 1) * 512],
                            start=(d == 0), stop=(d == DT - 1),
                        )
                    nc.vector.tensor_scalar(
                        out=qkT_t[pi * MT + mt][:, tc4 * 512 : (tc4 + 1) * 512],
                        in0=ps, scalar1=b_sb[:, mt : mt + 1], scalar2=None,
                        op0=mybir.AluOpType.add,
                    )

        # ---- phase 2b: v projection -> v_aug [t, 8*65] fp16 (ones col per head) ----
        v_t = [vpool.tile([128, NH * 65], F16, name=f"v{tt}") for tt in range(TT)]
        wvs = []
        for d in range(DT):
            wvd = wt.tile([128, M], F16)
            nc.sync.dma_start(out=wvd, in_=wvT_d[d * 128 : (d + 1) * 128, :])
            wvs.append(wvd)
        for tt in range(TT):
            nc.vector.memset(v_t[tt], 1.0)
            ps = psP.tile([128, 512], F32)
            for d in range(DT):
                nc.tensor.matmul(
                    ps,
                    lhsT=xcsT_t[d][:, tt * 128 : (tt + 1) * 128],
                    rhs=wvs[d],
                    start=(d == 0), stop=(d == DT - 1),
                )
            for h in range(NH):
                nc.vector.tensor_add(
                    out=v_t[tt][:, h * 65 : h * 65 + 64],
                    in0=ps[:, h * 64 : (h + 1) * 64],
                    in1=bv_sb[:, h * 64 : (h + 1) * 64],
                )

        # ---- phase 3: attention, head-pair at a time ----
        yT_t = [ypool.tile([128, T], F16, name=f"yT{i}") for i in range(MT)]
        for p in range(NPAIR):
            qa = qkT_t[p]
            ka = qkT_t[MT + p]
            for ic in range(4):
                njb = 4 * ic + 4
                U = [psUU.tile([128, 512], F32, name=f"U{hh}") for hh in range(2)]
                for g in range(njb // 2):
                    jbs = (2 * g, 2 * g + 1)
                    Sh = [
                        psSA.tile([128, 1024], F32, name="SA"),
                        psSB.tile([128, 1024], F32, name="SB"),
                    ]
                    PTh = []
                    for hh in range(2):
                        ro = hh * 64
                        h = 2 * p + hh
                        S = Sh[hh]
                        for jj, jb in enumerate(jbs):
                            col0 = max(0, jb * 128 - ic * 512)
                            nc.tensor.matmul(
                                S[:, jj * 512 + col0 : (jj + 1) * 512],
                                lhsT=ka[ro : ro + 64, jb * 128 : (jb + 1) * 128],
                                rhs=qa[ro : ro + 64, ic * 512 + col0 : (ic + 1) * 512],
                                start=True, stop=True,
                            )
                    for hh in range(2):
                        h = 2 * p + hh
                        S = Sh[hh]
                        PT = ptp.tile([128, 1024], F16)
                        nc.scalar.activation(
                            out=PT, in_=S,
                            func=mybir.ActivationFunctionType.Exp,
                            bias=0.0, scale=1.0,
                        )
                        # rel-bias bands + causal mask via P *= exp(band)
                        # (exp(S+b) = exp(S)*exp(b); masked entries get 0)
                        for jj, jb in enumerate(jbs):
                            ibs = [ib for ib in (jb, jb + 1)
                                   if 4 * ic <= ib <= 4 * ic + 3]
                            if not ibs:
                                continue
                            c = jj * 512 + (ibs[0] - 4 * ic) * 128
                            bc = h * 256 + (ibs[0] - jb) * 128
                            w = 128 * len(ibs)
                            nc.vector.tensor_mul(
                                out=PT[:, c : c + w],
                                in0=PT[:, c : c + w],
                                in1=expbT_sb[:, bc : bc + w],
                            )
                        for jj, jb in enumerate(jbs):
                            col0 = max(0, jb * 128 - ic * 512)
                            if col0 > 0:
                                nc.vector.memset(PT[:, jj * 512 : jj * 512 + col0], 0.0)
                        PTh.append(PT)
                    for hh in range(2):
                        h = 2 * p + hh
                        for jj, jb in enumerate(jbs):
                            nc.tensor.matmul(
                                U[hh][0:65, :],
                                lhsT=v_t[jb][:, h * 65 : h * 65 + 65],
                                rhs=PTh[hh][:, jj * 512 : (jj + 1) * 512],
                                start=(jb == 0), stop=(jb == njb - 1),
                            )
                for hh in range(2):
                    ro = hh * 64
                    zc = zpool.tile([1, 512], F32)
                    nc.vector.tensor_copy(out=zc, in_=U[hh][64:65, :])
                    rz = zpool.tile([1, 512], F32)
                    nc.vector.reciprocal_approx_fast(out=rz, in_=zc)
                    rzb = zpool.tile([64, 512], F32)
                    nc.gpsimd.partition_broadcast(rzb, rz)
                    nc.vector.tensor_mul(
                        out=yT_t[p][ro : ro + 64, ic * 512 : (ic + 1) * 512],
                        in0=U[hh][0:64, :], in1=rzb,
                    )

        # ---- phase 4: output projection (partial; host adds residual) ----
        wos = []
        for kt in range(MT):
            wod = wopool.tile([128, D], F16)
            nc.sync.dma_start(out=wod, in_=woT_d[kt * 128 : (kt + 1) * 128, :])
            wos.append(wod)
        for tt in range(TT):
            for oc in range(2):
                ps = psP.tile([128, 512], F32)
                for kt in range(MT):
                    nc.tensor.matmul(
                        ps,
                        lhsT=yT_t[kt][:, tt * 128 : (tt + 1) * 128],
                        rhs=wos[kt][:, oc * 512 : (oc + 1) * 512],
                        start=(kt == 0), stop=(kt == MT - 1),
                    )
                osb = outp.tile([128, 512], F32)
                nc.vector.tensor_copy(out=osb, in_=ps)
                nc.sync.dma_start(
                    out=out_d[tt * 128 : (tt + 1) * 128, oc * 512 : (oc + 1) * 512],
                    in_=osb,
                )

    nc.compile()
    return nc


def _host_prep(inputs):
    """Build the 8 per-core input maps."""
    x = np.asarray(inputs["x"], dtype=np.float32)
    Wq = np.asarray(inputs["Wq"], dtype=np.float32)
    Wk = np.asarray(inputs["Wk"], dtype=np.float32)
    Wv = np.asarray(inputs["Wv"], dtype=np.float32)
    Wo = np.asarray(inputs["Wo"], dtype=np.float32)
    rel = np.asarray(inputs["rel"], dtype=np.float32)
    gamma = np.asarray(inputs["ln_gamma"], dtype=np.float32)
    beta = np.asarray(inputs["ln_beta"], dtype=np.float32)

    sc = 1.0 / math.sqrt(DH)
    ii = np.arange(128)
    dist = ii[None, :] - ii[:, None]  # [j, i] = i - j

    half = {}
    for hh in range(2):
        hs = slice(hh * M, (hh + 1) * M)
        Wq_h, Wk_h, Wv_h, Wo_h = Wq[hs], Wk[hs], Wv[hs], Wo[:, hs]
        wqT = np.ascontiguousarray((Wq_h * gamma[None, :] * sc).T).astype(np.float16)
        wkT = np.ascontiguousarray((Wk_h * gamma[None, :]).T).astype(np.float16)
        wvT = np.ascontiguousarray((Wv_h * gamma[None, :]).T).astype(np.float16)
        woT = np.ascontiguousarray(Wo_h.T).astype(np.float16)
        bq = ((Wq_h @ beta) * sc).reshape(MT, 128).T.astype(np.float32)
        bk = (Wk_h @ beta).reshape(MT, 128).T.astype(np.float32)
        bv = np.tile((Wv_h @ beta)[None, :], (128, 1)).astype(np.float16)

        # exp of transposed bias bands, [j, i] layout; the far-field constant
        # rel[g,128] is dropped (softmax shift-invariance), causal mask is an
        # exact 0 factor
        expbT = np.zeros((128, NH * 256), dtype=np.float32)
        for h in range(NH):
            g = hh * NH + h
            r128 = rel[g, 128]
            bT0 = np.where(
                dist >= 0, rel[g, np.clip(dist, 0, 128)] - r128, -np.inf
            )
            bT1 = rel[g, np.minimum(128 + dist, 128)] - r128
            expbT[:, h * 256 : h * 256 + 128] = np.exp(bT0)
            expbT[:, h * 256 + 128 : h * 256 + 256] = np.exp(bT1)
        half[hh] = dict(
            wqT=wqT, wkT=wkT, wvT=wvT, woT=woT,
            bq=np.ascontiguousarray(bq), bk=np.ascontiguousarray(bk), bv=bv,
            expbT=expbT.astype(np.float16),
            ident=np.eye(128, dtype=np.float16),
        )

    in_maps = []
    for c in range(NCORES):
        b, hh = c // 2, c % 2
        m = dict(half[hh])
        m["x"] = np.ascontiguousarray(x[b])
        in_maps.append(m)
    return in_maps, x


def kernel(**inputs) -> np.ndarray:
    global _CACHED_NC
    if _CACHED_NC is None:
        _CACHED_NC = build_nc()
    nc = _CACHED_NC
    in_maps, x = _host_prep(inputs)
    res = run_bass_kernel_spmd(nc, in_maps, core_ids=list(range(NCORES)))
    out = np.empty_like(x)
    for b in range(4):
        out[b] = x[b] + res.results[2 * b]["out"] + res.results[2 * b + 1]["out"]
    return out


if __name__ == "__main__":
    rng = np.random.default_rng(0)
    fake = {
        "x": rng.standard_normal((4, T, D), dtype=np.float32),
        "Wq": rng.standard_normal((D, D), dtype=np.float32) / 32,
        "Wk": rng.standard_normal((D, D), dtype=np.float32) / 32,
        "Wv": rng.standard_normal((D, D), dtype=np.float32) / 32,
        "Wo": rng.standard_normal((D, D), dtype=np.float32) / 32,
        "rel": np.tile(np.linspace(0, -2, 129, dtype=np.float32), (16, 1)),
        "ln_gamma": np.ones(D, np.float32),
        "ln_beta": np.zeros(D, np.float32),
    }
    y = kernel(**fake)
    print("ran ok", y.shape, y.dtype)
